# revision 3
# baseline (speedup 1.0000x reference)
"""DeepSeek-V3.2 MLA attention on 8 Trainium2 NeuronCores (Bass/Tile).

Strategy (tensor parallel over heads, per the sharding hint):
  Launch A: sequence-sharded latent projections. Core c computes the
    q/kv down-projections + RMSNorm for its 256-token slice, in
    feature-major ("transposed") layout so no on-chip transposes are
    needed anywhere. Host gathers the 8 slices. All operands bf16,
    host-pretiled so every DMA is contiguous.
  Launch B: head-sharded attention. Core c owns heads (2c, 2c+1): up-
    projections, Q@K^T (computed transposed: [k, q]), mask add, exp,
    denominator via DVE accumulate + ones-matmul, P@V, o-proj partial.
    Host sums the 8 partial outputs (the all-reduce after o_proj).
    The score/PV loop only visits k-tiles that are not fully masked
    out (host classifies each 128x512 mask block as pass-through /
    additive / fully-masked and the kernel is built for exactly that
    structure) - for the causal mask this skips the 37.5% of attention
    work above the diagonal and all mask DMA except the diagonal band.

Host-side precomputation folds gqa/gkva into Wqb/Wkvb rows, the softmax
1/sqrt(192) into the q-latent normalization, and transposes/retiles
all tensors (layout prep only - all FLOPs of the module run on device).
"""

import numpy as np

import concourse.bass as bass
import concourse.tile as tile
from concourse import bacc, mybir
from concourse.bass_utils import run_bass_kernel_spmd

F32 = mybir.dt.float32
F32R = mybir.dt.float32r
BF16 = mybir.dt.bfloat16

S = 2048
HID = 2048
QL = 1536
KVL = 512
ROPE = 64
NOPE = 128
VH = 128
NH = 16
NCORES = 8
HPC = NH // NCORES          # heads per core = 2
SL = S // NCORES            # token slice per core in launch A = 256
QLT = QL // 128             # 12
KVT = KVL // 128            # 4
HT = HID // 128             # 16
ST = S // 128               # 16
QC = 512                    # attention query chunk
NQC = S // QC               # 4
CH = 512                    # up-projection chunk (moving dim)
NCH = S // CH               # 4
EPS = 1e-6

_CACHE = {}


def _build_a():
    """Launch A: latents for a 256-token slice, feature-major, bf16 io.

    in : hsl [128, HT*SL] (hidden^T slice, tiled), wqa [128, QLT*HT*128],
         wkva [128, KVT*HT*128], wkvr [128, HT*ROPE]   (all bf16, pretiled)
    out: q_lat [QL, SL]  = rmsnorm(hidden@Wqa) / sqrt(192)  (g folded later)
         kv_lat [KVL, SL] = rmsnorm-normalized kv latent
         rp_lat [ROPE, SL] = raw shared k_rope
    """
    nc = bacc.Bacc("TRN2", target_bir_lowering=False, debug=False,
                   num_devices=NCORES)
    hsl = nc.dram_tensor("hsl", [128, HT * SL], BF16,
                         kind="ExternalInput").ap()
    wqa = nc.dram_tensor("wqa", [128, QLT * HT * 128], BF16,
                         kind="ExternalInput").ap()
    wkva = nc.dram_tensor("wkva", [128, KVT * HT * 128], BF16,
                          kind="ExternalInput").ap()
    wkvr = nc.dram_tensor("wkvr", [128, HT * ROPE], BF16,
                          kind="ExternalInput").ap()
    q_lat = nc.dram_tensor("q_lat", [QL, SL], BF16,
                           kind="ExternalOutput").ap()
    kv_lat = nc.dram_tensor("kv_lat", [KVL, SL], BF16,
                            kind="ExternalOutput").ap()
    rp_lat = nc.dram_tensor("rp_lat", [ROPE, SL], BF16,
                            kind="ExternalOutput").ap()

    with tile.TileContext(nc) as tc:
        with tc.tile_pool(name="w", bufs=1) as wp, \
             tc.tile_pool(name="h", bufs=1) as hp, \
             tc.tile_pool(name="lat", bufs=1) as lp, \
             tc.tile_pool(name="tmp", bufs=3) as tp, \
             tc.tile_pool(name="ps", bufs=2, space="PSUM") as pp, \
             tc.tile_pool(name="pss", bufs=2, space="PSUM") as psp:
            ht = hp.tile([128, HT * SL], BF16, tag="ht")
            nc.sync.dma_start(ht[:], hsl[:, :])
            htt = [ht[:, j * SL:(j + 1) * SL] for j in range(HT)]
            wqa_s = wp.tile([128, QLT * HT * 128], BF16, tag="wqa")
            for m in range(QLT):
                nc.sync.dma_start(
                    wqa_s[:, m * HT * 128:(m + 1) * HT * 128],
                    wqa[:, m * HT * 128:(m + 1) * HT * 128])
            wkva_s = wp.tile([128, KVT * HT * 128], BF16, tag="wkva")
            for m in range(KVT):
                nc.sync.dma_start(
                    wkva_s[:, m * HT * 128:(m + 1) * HT * 128],
                    wkva[:, m * HT * 128:(m + 1) * HT * 128])
            wkvr_s = wp.tile([128, HT * ROPE], BF16, tag="wkvr")
            nc.sync.dma_start(wkvr_s[:], wkvr[:, :])

            ones_f = wp.tile([128, 1], F32, tag="ones")
            nc.vector.memset(ones_f[:], 1.0)
            ones = ones_f[:].bitcast(F32R)
            epsq = wp.tile([1, 1], F32, tag="epsq")
            nc.vector.memset(epsq[:], 192.0 * EPS)
            epsk = wp.tile([1, 1], F32, tag="epsk")
            nc.vector.memset(epsk[:], EPS)

            def down_path(n_tiles, col_of, ssq_scale, eps_ap, out_dram, pfx):
                """Shared q/kv path: down-proj, ssq, rsqrt, normalize, store."""
                raw = []
                ps_ssq = psp.tile([1, SL], F32, tag="ssq")
                for m in range(n_tiles):
                    ps = pp.tile([128, SL], F32, tag="dps")
                    for j in range(HT):
                        nc.tensor.matmul(ps[:], col_of(j, m), htt[j],
                                         start=(j == 0), stop=(j == HT - 1))
                    r = lp.tile([128, SL], F32R, tag=f"raw{pfx}{m}")
                    nc.vector.tensor_copy(r[:], ps[:])
                    raw.append(r)
                    sq = tp.tile([128, SL], F32R, tag="sq")
                    nc.scalar.square(sq[:], ps[:])
                    nc.tensor.matmul(ps_ssq[:], ones, sq[:],
                                     start=(m == 0), stop=(m == n_tiles - 1))
                sd = tp.tile([1, SL], F32, tag="sd")
                nc.scalar.activation(sd[:], ps_ssq[:],
                                     mybir.ActivationFunctionType.Sqrt,
                                     bias=eps_ap[:], scale=ssq_scale)
                rr = tp.tile([1, SL], F32, tag="rr")
                nc.vector.reciprocal_approx_fast(rr[:], sd[:])
                rb = tp.tile([128, SL], F32, tag="rb")
                nc.gpsimd.partition_broadcast(rb[:], rr[:1])
                for m in range(n_tiles):
                    no = tp.tile([128, SL], BF16, tag="no")
                    nc.vector.tensor_mul(no[:], raw[m][:], rb[:])
                    nc.sync.dma_start(out_dram[m * 128:(m + 1) * 128, :],
                                      no[:])

            # q: fold softmax scale 1/sqrt(192) into the rmsnorm scale:
            #   r = 1/sqrt(192*(ssq/QL + eps)) = 1/sqrt(ssq*(192/QL) + 192*eps)
            down_path(QLT, lambda j, m: wqa_s[:, (m * HT + j) * 128:
                                              (m * HT + j + 1) * 128],
                      192.0 / QL, epsq, q_lat, "q")
            down_path(KVT, lambda j, m: wkva_s[:, (m * HT + j) * 128:
                                               (m * HT + j + 1) * 128],
                      1.0 / KVL, epsk, kv_lat, "k")
            # raw shared rope part (no norm)
            ps = pp.tile([64, SL], F32, tag="rps")
            for j in range(HT):
                nc.tensor.matmul(
                    ps[:], wkvr_s[:, j * ROPE:(j + 1) * ROPE],
                    htt[j], start=(j == 0), stop=(j == HT - 1))
            ro = tp.tile([64, SL], BF16, tag="ro")
            nc.vector.tensor_copy(ro[:], ps[:])
            nc.sync.dma_start(rp_lat[:, :], ro[:])
    nc.compile()
    return nc


def _build_b(struct, n_mask):
    """Launch B: 2 heads of attention + o-proj partial over the full seq.

    struct: per query-chunk tuple of (kt, mask_idx) with mask_idx=-1 for
    pass-through tiles; fully-masked k-tiles are absent.

    in : qlat [128, NCH*QLT*CH] (chunk-major latents), kvlat [128,
         NCH*KVT*CH], kr0/kr1 [128, S] (head-padded shared rope keys),
         mask [128, n_mask*QC], wqn [128, QLT*HPC*NOPE],
         wqr [128, QLT*HPC*64], wkn [128, KVT*HPC*NOPE],
         wkv [128, KVT*HPC*VH], wo [128, HPC*HID]     (all bf16)
    out: part [S, HID] bf16 (this core's 2-head contribution)
    """
    nc = bacc.Bacc("TRN2", target_bir_lowering=False, debug=False,
                   num_devices=NCORES)
    qlat = nc.dram_tensor("qlat", [128, NCH * QLT * CH], BF16,
                          kind="ExternalInput").ap()
    kvlat = nc.dram_tensor("kvlat", [128, NCH * KVT * CH], BF16,
                           kind="ExternalInput").ap()
    kr0 = nc.dram_tensor("kr0", [128, S], BF16, kind="ExternalInput").ap()
    kr1 = nc.dram_tensor("kr1", [128, S], BF16, kind="ExternalInput").ap()
    mask = nc.dram_tensor("mask", [128, max(n_mask, 1) * QC], BF16,
                          kind="ExternalInput").ap()
    wqn = nc.dram_tensor("wqn", [128, QLT * HPC * NOPE], BF16,
                         kind="ExternalInput").ap()
    wqr = nc.dram_tensor("wqr", [128, QLT * HPC * 64], BF16,
                         kind="ExternalInput").ap()
    wkn = nc.dram_tensor("wkn", [128, KVT * HPC * NOPE], BF16,
                         kind="ExternalInput").ap()
    wkv = nc.dram_tensor("wkv", [128, KVT * HPC * VH], BF16,
                         kind="ExternalInput").ap()
    wo = nc.dram_tensor("wo", [128, HPC * HID], BF16,
                        kind="ExternalInput").ap()
    part = nc.dram_tensor("part", [S, HID], BF16, kind="ExternalOutput").ap()

    with tile.TileContext(nc) as tc:
        with tc.tile_pool(name="w", bufs=1) as wp, \
             tc.tile_pool(name="act", bufs=1) as ap_, \
             tc.tile_pool(name="lq", bufs=2) as lqp, \
             tc.tile_pool(name="et", bufs=4) as ep, \
             tc.tile_pool(name="es", bufs=3) as esp, \
             tc.tile_pool(name="tmp", bufs=2) as tp, \
             tc.tile_pool(name="out", bufs=3) as op, \
             tc.tile_pool(name="ps", bufs=3, space="PSUM") as pp, \
             tc.tile_pool(name="pso", bufs=3, space="PSUM") as pop, \
             tc.tile_pool(name="psden", bufs=2, space="PSUM") as pdp:
            ones_f = wp.tile([128, 1], F32, tag="ones")
            nc.vector.memset(ones_f[:], 1.0)
            ones = ones_f[:].bitcast(F32R)
            zb = wp.tile([128, 1], F32, tag="zb")
            nc.vector.memset(zb[:], 0.0)

            # ---- persistent per-head activations (feature-major) ----
            qn_T = [ap_.tile([128, S], BF16, tag=f"qnT{h}", name=f"qnT{h}")
                    for h in range(HPC)]
            qr2_T = ap_.tile([128, S], BF16, tag="qr2T")
            kn_T = [ap_.tile([128, S], BF16, tag=f"knT{h}", name=f"knT{h}")
                    for h in range(HPC)]
            v2 = ap_.tile([128, ST * HPC * VH], BF16, tag="v2")
            krs = [ap_.tile([128, S], BF16, tag=f"krs{h}", name=f"krs{h}")
                   for h in range(HPC)]
            ot = [ap_.tile([128, QC], BF16, tag=f"ot{i}", name=f"ot{i}")
                  for i in range(NQC * HPC)]

            def load_chunk(c):
                lq = lqp.tile([128, QLT * CH], BF16, tag="lq", name="lq")
                nc.sync.dma_start(lq[:],
                                  qlat[:, c * QLT * CH:(c + 1) * QLT * CH])
                lk = lqp.tile([128, KVT * CH], BF16, tag="lk", name="lk")
                nc.sync.dma_start(lk[:],
                                  kvlat[:, c * KVT * CH:(c + 1) * KVT * CH])
                return lq, lk

            pend = load_chunk(0)
            # ---- weights / rope keys / mask band to SBUF ----
            nc.sync.dma_start(krs[0][:], kr0[:, :])
            nc.sync.dma_start(krs[1][:], kr1[:, :])
            mt = wp.tile([128, max(n_mask, 1) * QC], BF16, tag="mask")
            nc.sync.dma_start(mt[:], mask[:, :])
            wqn_s = wp.tile([128, QLT * HPC * NOPE], BF16, tag="wqn")
            nc.sync.dma_start(wqn_s[:], wqn[:, :])
            wqr_s = wp.tile([128, QLT * HPC * 64], BF16, tag="wqr")
            nc.sync.dma_start(wqr_s[:], wqr[:, :])
            wkn_s = wp.tile([128, KVT * HPC * NOPE], BF16, tag="wkn")
            nc.sync.dma_start(wkn_s[:], wkn[:, :])
            wkv_s = wp.tile([128, KVT * HPC * VH], BF16, tag="wkv")
            nc.sync.dma_start(wkv_s[:], wkv[:, :])
            wo_s = wp.tile([128, HPC * HID], BF16, tag="wo")
            nc.sync.dma_start(wo_s[:], wo[:, :])

            # ---- phase 1: up-projections, chunked over tokens ----
            for c in range(NCH):
                csl = slice(c * CH, (c + 1) * CH)
                lq, lk = pend
                if c + 1 < NCH:
                    pend = load_chunk(c + 1)

                for h in range(HPC):
                    ps = pp.tile([128, CH], F32, tag="ups")
                    for m in range(QLT):
                        nc.tensor.matmul(
                            ps[:],
                            wqn_s[:, m * HPC * NOPE + h * NOPE:
                                  m * HPC * NOPE + (h + 1) * NOPE],
                            lq[:, m * CH:(m + 1) * CH],
                            start=(m == 0), stop=(m == QLT - 1))
                    nc.vector.tensor_copy(qn_T[h][:, csl], ps[:])
                ps = pp.tile([128, CH], F32, tag="ups")
                for m in range(QLT):
                    nc.tensor.matmul(ps[:],
                                     wqr_s[:, m * HPC * 64:(m + 1) * HPC * 64],
                                     lq[:, m * CH:(m + 1) * CH],
                                     start=(m == 0), stop=(m == QLT - 1))
                nc.vector.tensor_copy(qr2_T[:, csl], ps[:])
                for h in range(HPC):
                    ps = pp.tile([128, CH], F32, tag="ups")
                    for m in range(KVT):
                        nc.tensor.matmul(
                            ps[:],
                            wkn_s[:, m * HPC * NOPE + h * NOPE:
                                  m * HPC * NOPE + (h + 1) * NOPE],
                            lk[:, m * CH:(m + 1) * CH],
                            start=(m == 0), stop=(m == KVT - 1))
                    nc.vector.tensor_copy(kn_T[h][:, csl], ps[:])
                for st in range(CH // 128):
                    ps = pp.tile([128, HPC * VH], F32, tag="ups")
                    for m in range(KVT):
                        nc.tensor.matmul(
                            ps[:],
                            lk[:, m * CH + st * 128:m * CH + (st + 1) * 128],
                            wkv_s[:, m * HPC * VH:(m + 1) * HPC * VH],
                            start=(m == 0), stop=(m == KVT - 1))
                    gst = c * (CH // 128) + st
                    nc.vector.tensor_copy(
                        v2[:, gst * HPC * VH:(gst + 1) * HPC * VH], ps[:])

            # ---- phase 2: attention per query chunk, heads interleaved ----
            for qc in range(NQC):
                qsl = slice(qc * QC, (qc + 1) * QC)
                tiles = struct[qc]
                nk = len(tiles)
                ps_o = [pop.tile([128, QC], F32, tag="po", name=f"po{h}")
                        for h in range(HPC)]
                esum = [esp.tile([128, QC], F32R, tag="esum",
                                 name=f"esum{h}") for h in range(HPC)]
                for ki, (kt, midx) in enumerate(tiles):
                    ksl = slice(kt * 128, (kt + 1) * 128)
                    for h in range(HPC):
                        ps_s = pp.tile([128, QC], F32, tag="ups",
                                       name="ps_s")
                        nc.tensor.matmul(ps_s[:], kn_T[h][:, ksl],
                                         qn_T[h][:, qsl],
                                         start=True, stop=False)
                        nc.tensor.matmul(ps_s[:], krs[h][:, ksl],
                                         qr2_T[:, qsl],
                                         start=False, stop=True)
                        if midx >= 0:
                            nc.vector.tensor_add(
                                ps_s[:], ps_s[:],
                                mt[:, midx * QC:(midx + 1) * QC])
                        et = ep.tile([128, QC], BF16, tag="et", name="et")
                        nc.scalar.activation(
                            et[:], ps_s[:],
                            mybir.ActivationFunctionType.Exp,
                            bias=zb[:], scale=1.0)
                        if ki == 0:
                            nc.vector.tensor_copy(esum[h][:], et[:])
                        else:
                            nc.vector.tensor_add(esum[h][:], esum[h][:],
                                                 et[:])
                        nc.tensor.matmul(
                            ps_o[h][:],
                            v2[:, kt * HPC * VH + h * VH:
                               kt * HPC * VH + (h + 1) * VH],
                            et[:], start=(ki == 0), stop=(ki == nk - 1))
                ps_den = []
                for h in range(HPC):
                    pd = pdp.tile([1, QC], F32, tag="den", name=f"den{h}")
                    nc.tensor.matmul(pd[:], ones, esum[h][:],
                                     start=True, stop=True)
                    ps_den.append(pd)
                for h in range(HPC):
                    dencp = tp.tile([1, QC], F32, tag="dencp")
                    nc.scalar.copy(dencp[:], ps_den[h][:])
                    rd = tp.tile([1, QC], F32, tag="rd")
                    nc.vector.reciprocal_approx_fast(rd[:], dencp[:])
                    rdb = tp.tile([128, QC], F32, tag="rdb")
                    nc.gpsimd.partition_broadcast(rdb[:], rd[:1])
                    nc.vector.tensor_mul(ot[qc * HPC + h][:], ps_o[h][:],
                                         rdb[:])

            # ---- phase 3: o-proj from all stored attention outputs ----
            for qc in range(NQC):
                for st in range(QC // 128):
                    for nn in range(HID // 512):
                        ps_f = pop.tile([128, 512], F32, tag="po",
                                        name="ps_f")
                        for h in range(HPC):
                            nc.tensor.matmul(
                                ps_f[:],
                                ot[qc * HPC + h][:, st * 128:(st + 1) * 128],
                                wo_s[:, h * HID + nn * 512:
                                     h * HID + (nn + 1) * 512],
                                start=(h == 0), stop=(h == HPC - 1))
                        fo = op.tile([128, 512], BF16, tag="fo")
                        nc.scalar.copy(fo[:], ps_f[:])
                        nc.sync.dma_start(
                            part[qc * QC + st * 128:qc * QC + (st + 1) * 128,
                                 nn * 512:(nn + 1) * 512], fo[:])
    nc.compile()
    return nc


def _get_a():
    if "a" not in _CACHE:
        _CACHE["a"] = _build_a()
    return _CACHE["a"]


def _get_b(struct, n_mask):
    key = ("b", struct, n_mask)
    if key not in _CACHE:
        _CACHE[key] = _build_b(struct, n_mask)
    return _CACHE[key]


def _analyze_mask(mask_qk):
    """Classify each (qc, kt) block of mask[q, k]; dedupe mixed tiles.

    Returns (struct, mask_tiles [128, n*QC] bf16-able f32 array).
    struct[qc] = tuple of (kt, mask_idx) for k-tiles to visit;
    mask_idx -1 means no mask add needed; fully-masked tiles dropped.
    """
    import ml_dtypes
    uniq = {}
    tiles = []
    struct = []
    for qc in range(NQC):
        row = []
        for kt in range(ST):
            blk = mask_qk[qc * QC:(qc + 1) * QC, kt * 128:(kt + 1) * 128]
            if np.all(blk <= -1e8):
                continue
            if np.all(blk == 0.0):
                row.append((kt, -1))
                continue
            t = np.ascontiguousarray(blk.T).astype(ml_dtypes.bfloat16)
            key = t.tobytes()
            if key not in uniq:
                uniq[key] = len(tiles)
                tiles.append(t)
            row.append((kt, uniq[key]))
        struct.append(tuple(row))
    if tiles:
        mask_tiles = np.concatenate(tiles, axis=1)
    else:
        mask_tiles = np.zeros((128, QC), dtype=ml_dtypes.bfloat16)
    return tuple(struct), mask_tiles


def _prep(hidden_states, attention_mask, Wqa, gqa, Wqb, Wkva, gkva, Wkvb, Wo):
    import ml_dtypes
    bf = ml_dtypes.bfloat16
    f = np.float32
    hid_T = np.ascontiguousarray(hidden_states[0].T, dtype=f)  # [HID, S]
    struct, mask_tiles = _analyze_mask(np.asarray(attention_mask[0, 0], f))
    Wqb_g = (np.asarray(gqa, f)[:, None] * np.asarray(Wqb, f)).astype(f)
    Wkvb_g = (np.asarray(gkva, f)[:, None] * np.asarray(Wkvb, f)).astype(f)

    Wqa_f = np.asarray(Wqa, f)
    wqa_t = Wqa_f.reshape(HT, 128, QLT, 128).transpose(1, 2, 0, 3) \
        .reshape(128, QLT * HT * 128).astype(bf)
    Wkva_f = np.asarray(Wkva, f)
    wkva_t = Wkva_f[:, :KVL].reshape(HT, 128, KVT, 128) \
        .transpose(1, 2, 0, 3).reshape(128, KVT * HT * 128).astype(bf)
    wkvr_t = Wkva_f[:, KVL:].reshape(HT, 128, ROPE).transpose(1, 0, 2) \
        .reshape(128, HT * ROPE).astype(bf)

    ins_a, ins_b = [], []
    for c in range(NCORES):
        hsl = hid_T[:, c * SL:(c + 1) * SL]  # [HID, SL]
        hsl_t = hsl.reshape(HT, 128, SL).transpose(1, 0, 2) \
            .reshape(128, HT * SL).astype(bf)
        ins_a.append({
            "hsl": np.ascontiguousarray(hsl_t),
            "wqa": wqa_t, "wkva": wkva_t, "wkvr": wkvr_t,
        })
        heads = [HPC * c + h for h in range(HPC)]
        wqn = np.concatenate([Wqb_g[:, h * 192:h * 192 + NOPE]
                              for h in heads], axis=1)      # [QL, 256]
        wqr = np.concatenate([Wqb_g[:, h * 192 + NOPE:(h + 1) * 192]
                              for h in heads], axis=1)      # [QL, 128]
        wkn = np.concatenate([Wkvb_g[:, h * 256:h * 256 + NOPE]
                              for h in heads], axis=1)      # [KVL, 256]
        wkv = np.concatenate([Wkvb_g[:, h * 256 + NOPE:(h + 1) * 256]
                              for h in heads], axis=1)      # [KVL, 256]
        wo = np.concatenate([np.asarray(Wo, f)[h * VH:(h + 1) * VH, :]
                             for h in heads], axis=0)       # [256, HID]
        ins_b.append({
            "mask": mask_tiles,
            "wqn": wqn.reshape(QLT, 128, HPC * NOPE).transpose(1, 0, 2)
                      .reshape(128, QLT * HPC * NOPE).astype(bf),
            "wqr": wqr.reshape(QLT, 128, HPC * 64).transpose(1, 0, 2)
                      .reshape(128, QLT * HPC * 64).astype(bf),
            "wkn": wkn.reshape(KVT, 128, HPC * NOPE).transpose(1, 0, 2)
                      .reshape(128, KVT * HPC * NOPE).astype(bf),
            "wkv": wkv.reshape(KVT, 128, HPC * VH).transpose(1, 0, 2)
                      .reshape(128, KVT * HPC * VH).astype(bf),
            "wo": wo.reshape(HPC, 128, HID).transpose(1, 0, 2)
                    .reshape(128, HPC * HID).astype(bf),
        })
    n_mask = mask_tiles.shape[1] // QC
    return ins_a, ins_b, struct, n_mask


def _run(ins_a, ins_b, struct, n_mask, trace=False):
    import ml_dtypes
    bf = ml_dtypes.bfloat16
    core_ids = list(range(NCORES))
    res_a = run_bass_kernel_spmd(_get_a(), ins_a, core_ids, trace=trace)
    qlat = np.concatenate([res_a.results[c]["q_lat"] for c in range(NCORES)],
                          axis=1)                            # [QL, S] bf16
    kvlat = np.concatenate([res_a.results[c]["kv_lat"]
                            for c in range(NCORES)], axis=1)  # [KVL, S]
    rplat = np.concatenate([res_a.results[c]["rp_lat"]
                            for c in range(NCORES)], axis=1)  # [ROPE, S]
    qlat_t = np.ascontiguousarray(
        qlat.reshape(QLT, 128, NCH, CH).transpose(1, 2, 0, 3)
        .reshape(128, NCH * QLT * CH))
    kvlat_t = np.ascontiguousarray(
        kvlat.reshape(KVT, 128, NCH, CH).transpose(1, 2, 0, 3)
        .reshape(128, NCH * KVT * CH))
    kr0 = np.zeros((128, S), dtype=bf)
    kr0[:ROPE] = rplat
    kr1 = np.zeros((128, S), dtype=bf)
    kr1[ROPE:] = rplat
    for m in ins_b:
        m["qlat"] = qlat_t
        m["kvlat"] = kvlat_t
        m["kr0"] = kr0
        m["kr1"] = kr1
    res_b = run_bass_kernel_spmd(_get_b(struct, n_mask), ins_b, core_ids,
                                 trace=trace)
    out = res_b.results[0]["part"].astype(np.float32)
    for c in range(1, NCORES):
        out = out + res_b.results[c]["part"].astype(np.float32)
    return out[None], res_a, res_b


def kernel(hidden_states, attention_mask, Wqa, gqa, Wqb, Wkva, gkva, Wkvb, Wo):
    ins_a, ins_b, struct, n_mask = _prep(hidden_states, attention_mask, Wqa,
                                         gqa, Wqb, Wkva, gkva, Wkvb, Wo)
    out, _, _ = _run(ins_a, ins_b, struct, n_mask)
    return out


# revision 12
# speedup vs baseline: 1.1320x; 1.1320x over previous
"""DeepSeek-V3.2 MLA attention on 8 Trainium2 NeuronCores (Bass/Tile).

Strategy (tensor parallel over heads, per the sharding hint):
  Launch A: sequence-sharded latent projections. Core c computes the
    q/kv down-projections + RMSNorm for its 256-token slice, in
    feature-major ("transposed") layout so no on-chip transposes are
    needed anywhere. Host gathers the 8 slices. All operands bf16,
    host-pretiled so every DMA is contiguous.
  Launch B: head-sharded attention. Core c owns heads (2c, 2c+1): up-
    projections, Q@K^T (computed transposed: [k, q]), mask add, exp,
    denominator via DVE accumulate + ones-matmul, P@V, o-proj partial.
    Host sums the 8 partial outputs (the all-reduce after o_proj).
    The score/PV loop only visits k-tiles that are not fully masked
    out (host classifies each 128x512 mask block as pass-through /
    additive / fully-masked and the kernel is built for exactly that
    structure) - for the causal mask this skips the 37.5% of attention
    work above the diagonal and all mask DMA except the diagonal band.

Host-side precomputation folds gqa/gkva into Wqb/Wkvb rows, the softmax
1/sqrt(192) into the q-latent normalization, and transposes/retiles
all tensors (layout prep only - all FLOPs of the module run on device).
"""

import numpy as np

import concourse.bass as bass
import concourse.tile as tile
from concourse import bacc, mybir
from concourse.bass_utils import run_bass_kernel_spmd

F32 = mybir.dt.float32
F32R = mybir.dt.float32r
BF16 = mybir.dt.bfloat16

S = 2048
HID = 2048
QL = 1536
KVL = 512
ROPE = 64
NOPE = 128
VH = 128
NH = 16
NCORES = 8
HPC = NH // NCORES          # heads per core = 2
SL = S // NCORES            # token slice per core in launch A = 256
QLT = QL // 128             # 12
KVT = KVL // 128            # 4
HT = HID // 128             # 16
ST = S // 128               # 16
QC = 512                    # attention query chunk
NQC = S // QC               # 4
CH = 512                    # up-projection chunk (moving dim)
NCH = S // CH               # 4
EPS = 1e-6

_CACHE = {}


def _build_a():
    """Launch A: latents for a 256-token slice, feature-major, bf16 io.

    in : hsl [128, HT*SL] (hidden^T slice, tiled), wqa [128, QLT*HT*128],
         wkva [128, KVT*HT*128], wkvr [128, HT*ROPE]   (all bf16, pretiled)
    out: q_lat [QL, SL]  = rmsnorm(hidden@Wqa) / sqrt(192)  (g folded later)
         kv_lat [KVL, SL] = rmsnorm-normalized kv latent
         rp_lat [ROPE, SL] = raw shared k_rope
    """
    nc = bacc.Bacc("TRN2", target_bir_lowering=False, debug=False,
                   num_devices=NCORES)
    hsl = nc.dram_tensor("hsl", [128, HT * SL], BF16,
                         kind="ExternalInput").ap()
    wqa = nc.dram_tensor("wqa", [128, QLT * HT * 128], BF16,
                         kind="ExternalInput").ap()
    wkva = nc.dram_tensor("wkva", [128, KVT * HT * 128], BF16,
                          kind="ExternalInput").ap()
    wkvr = nc.dram_tensor("wkvr", [128, HT * ROPE], BF16,
                          kind="ExternalInput").ap()
    q_lat = nc.dram_tensor("q_lat", [128, QLT * SL], BF16,
                           kind="ExternalOutput").ap()
    kv_lat = nc.dram_tensor("kv_lat", [128, KVT * SL], BF16,
                            kind="ExternalOutput").ap()
    rp_lat = nc.dram_tensor("rp_lat", [ROPE, SL], BF16,
                            kind="ExternalOutput").ap()

    with tile.TileContext(nc) as tc:
        with tc.tile_pool(name="w", bufs=1) as wp, \
             tc.tile_pool(name="h", bufs=1) as hp, \
             tc.tile_pool(name="lat", bufs=1) as lp, \
             tc.tile_pool(name="stg", bufs=1) as sp, \
             tc.tile_pool(name="tmp", bufs=3) as tp, \
             tc.tile_pool(name="ps", bufs=2, space="PSUM") as pp, \
             tc.tile_pool(name="pss", bufs=2, space="PSUM") as psp, \
             tc.tile_pool(name="psb", bufs=2, space="PSUM") as pbp:
            # DMA priority order: wqa block m interleaved with the h tiles
            # so the first q matmul chain starts after ~0.6MB of traffic.
            ht = hp.tile([128, HT * SL], BF16, tag="ht")
            wqa_s = wp.tile([128, QLT * HT * 128], BF16, tag="wqa")
            nc.sync.dma_start(wqa_s[:, :HT * 128], wqa[:, :HT * 128])
            for j in range(HT):
                nc.sync.dma_start(ht[:, j * SL:(j + 1) * SL],
                                  hsl[:, j * SL:(j + 1) * SL])
            for m in range(1, QLT):
                nc.sync.dma_start(
                    wqa_s[:, m * HT * 128:(m + 1) * HT * 128],
                    wqa[:, m * HT * 128:(m + 1) * HT * 128])
            htt = [ht[:, j * SL:(j + 1) * SL] for j in range(HT)]
            wkva_s = wp.tile([128, KVT * HT * 128], BF16, tag="wkva")
            for m in range(KVT):
                nc.sync.dma_start(
                    wkva_s[:, m * HT * 128:(m + 1) * HT * 128],
                    wkva[:, m * HT * 128:(m + 1) * HT * 128])
            wkvr_s = wp.tile([128, HT * ROPE], BF16, tag="wkvr")
            nc.sync.dma_start(wkvr_s[:], wkvr[:, :])

            ones_f = wp.tile([128, 1], F32, tag="ones")
            nc.vector.memset(ones_f[:], 1.0)
            ones = ones_f[:].bitcast(F32R)
            onesr_f = wp.tile([1, 128], F32, tag="onesr")
            nc.vector.memset(onesr_f[:], 1.0)
            onesr = onesr_f[:].bitcast(F32R)
            epsq = wp.tile([1, 1], F32, tag="epsq")
            nc.vector.memset(epsq[:], 192.0 * EPS)
            epsk = wp.tile([1, 1], F32, tag="epsk")
            nc.vector.memset(epsk[:], EPS)

            def down_path(n_tiles, col_of, ssq_scale, eps_ap, out_dram, pfx):
                """Shared q/kv path: down-proj, ssq, rsqrt, normalize, store."""
                raw = []
                ps_ssq = psp.tile([1, SL], F32, tag="ssq")
                for m in range(n_tiles):
                    ps = pp.tile([128, SL], F32, tag="dps")
                    for j in range(HT):
                        nc.tensor.matmul(ps[:], col_of(j, m), htt[j],
                                         start=(j == 0), stop=(j == HT - 1))
                    r = lp.tile([128, SL], F32R, tag=f"raw{pfx}{m}")
                    nc.vector.tensor_copy(r[:], ps[:])
                    raw.append(r)
                    sq = tp.tile([128, SL], F32R, tag="sq")
                    nc.scalar.square(sq[:], ps[:])
                    nc.tensor.matmul(ps_ssq[:], ones, sq[:],
                                     start=(m == 0), stop=(m == n_tiles - 1))
                sd = tp.tile([1, SL], F32, tag="sd")
                nc.scalar.activation(sd[:], ps_ssq[:],
                                     mybir.ActivationFunctionType.Sqrt,
                                     bias=eps_ap[:], scale=ssq_scale)
                rr = tp.tile([1, SL], F32, tag="rr")
                nc.vector.reciprocal_approx_fast(rr[:], sd[:])
                rrr = tp.tile([1, SL], F32R, tag="rrr")
                nc.vector.tensor_copy(rrr[:], rr[:])
                # broadcast 1/sd across partitions on the PE (rank-1 matmul)
                rb = pbp.tile([128, SL], F32, tag="rb")
                nc.tensor.matmul(rb[:], onesr, rrr[:], start=True, stop=True)
                stg = sp.tile([128, n_tiles * SL], BF16, tag=f"stg{pfx}")
                for m in range(n_tiles):
                    nc.vector.tensor_mul(stg[:, m * SL:(m + 1) * SL],
                                         raw[m][:], rb[:])
                nc.sync.dma_start(out_dram[:, :n_tiles * SL], stg[:])

            # q: fold softmax scale 1/sqrt(192) into the rmsnorm scale:
            #   r = 1/sqrt(192*(ssq/QL + eps)) = 1/sqrt(ssq*(192/QL) + 192*eps)
            down_path(QLT, lambda j, m: wqa_s[:, (m * HT + j) * 128:
                                              (m * HT + j + 1) * 128],
                      192.0 / QL, epsq, q_lat, "q")
            down_path(KVT, lambda j, m: wkva_s[:, (m * HT + j) * 128:
                                               (m * HT + j + 1) * 128],
                      1.0 / KVL, epsk, kv_lat, "k")
            # raw shared rope part (no norm)
            ps = pp.tile([64, SL], F32, tag="rps")
            for j in range(HT):
                nc.tensor.matmul(
                    ps[:], wkvr_s[:, j * ROPE:(j + 1) * ROPE],
                    htt[j], start=(j == 0), stop=(j == HT - 1))
            ro = tp.tile([64, SL], BF16, tag="ro")
            nc.vector.tensor_copy(ro[:], ps[:])
            nc.sync.dma_start(rp_lat[:, :], ro[:])
    nc.compile()
    return nc


def _build_b(struct, n_mask):
    """Launch B: 2 heads of attention + o-proj partial over the full seq.

    struct: per query-chunk tuple of (kt, mask_idx) with mask_idx=-1 for
    pass-through tiles; fully-masked k-tiles are absent.

    in : qlat [128, NCH*QLT*CH] (chunk-major latents), kvlat [128,
         NCH*KVT*CH], kr0/kr1 [128, S] (head-padded shared rope keys),
         mask [128, n_mask*QC], wqn [128, QLT*HPC*NOPE],
         wqr [128, QLT*HPC*64], wkn [128, KVT*HPC*NOPE],
         wkv [128, KVT*HPC*VH], wo [128, HPC*HID]     (all bf16)
    out: part [S, HID] bf16 (this core's 2-head contribution)
    """
    nc = bacc.Bacc("TRN2", target_bir_lowering=False, debug=False,
                   num_devices=NCORES)
    qlat = nc.dram_tensor("qlat", [128, NCH * QLT * CH], BF16,
                          kind="ExternalInput").ap()
    kvlat = nc.dram_tensor("kvlat", [128, NCH * KVT * CH], BF16,
                           kind="ExternalInput").ap()
    kr0 = nc.dram_tensor("kr0", [128, S], BF16, kind="ExternalInput").ap()
    kr1 = nc.dram_tensor("kr1", [128, S], BF16, kind="ExternalInput").ap()
    mask = nc.dram_tensor("mask", [128, max(n_mask, 1) * QC], BF16,
                          kind="ExternalInput").ap()
    wqn = nc.dram_tensor("wqn", [128, QLT * HPC * NOPE], BF16,
                         kind="ExternalInput").ap()
    wqr = nc.dram_tensor("wqr", [128, QLT * HPC * 64], BF16,
                         kind="ExternalInput").ap()
    wkn = nc.dram_tensor("wkn", [128, KVT * HPC * NOPE], BF16,
                         kind="ExternalInput").ap()
    wkv = nc.dram_tensor("wkv", [128, KVT * HPC * VH], BF16,
                         kind="ExternalInput").ap()
    wo = nc.dram_tensor("wo", [128, HPC * HID], BF16,
                        kind="ExternalInput").ap()
    part = nc.dram_tensor("part", [S, HID], BF16, kind="ExternalOutput").ap()

    with tile.TileContext(nc) as tc:
        with tc.tile_pool(name="w", bufs=1) as wp, \
             tc.tile_pool(name="act", bufs=1) as ap_, \
             tc.tile_pool(name="lq", bufs=2) as lqp, \
             tc.tile_pool(name="et", bufs=4) as ep, \
             tc.tile_pool(name="es", bufs=3) as esp, \
             tc.tile_pool(name="tmp", bufs=2) as tp, \
             tc.tile_pool(name="out", bufs=3) as op, \
             tc.tile_pool(name="ps", bufs=3, space="PSUM") as pp, \
             tc.tile_pool(name="pso", bufs=4, space="PSUM") as pop, \
             tc.tile_pool(name="psden", bufs=1, space="PSUM") as pdp:
            ones_b = wp.tile([128, 1], BF16, tag="ones")
            nc.vector.memset(ones_b[:], 1.0)
            zb = wp.tile([128, 1], F32, tag="zb")
            nc.vector.memset(zb[:], 0.0)

            # ---- persistent per-head activations (feature-major) ----
            qn_T = [ap_.tile([128, S], BF16, tag=f"qnT{h}", name=f"qnT{h}")
                    for h in range(HPC)]
            qr2_T = ap_.tile([128, S], BF16, tag="qr2T")
            kn_T = [ap_.tile([128, S], BF16, tag=f"knT{h}", name=f"knT{h}")
                    for h in range(HPC)]
            v2 = ap_.tile([128, ST * HPC * VH], BF16, tag="v2")
            krs = [ap_.tile([128, S], BF16, tag=f"krs{h}", name=f"krs{h}")
                   for h in range(HPC)]
            ot = [ap_.tile([128, QC], BF16, tag=f"ot{i}", name=f"ot{i}")
                  for i in range(NQC * HPC)]

            def load_chunk(c, lq=None, lk=None):
                if lq is None:
                    lq = lqp.tile([128, QLT * CH], BF16, tag="lq", name="lq")
                for m in range(QLT):
                    nc.sync.dma_start(
                        lq[:, m * CH:(m + 1) * CH],
                        qlat[:, (c * QLT + m) * CH:(c * QLT + m + 1) * CH])
                if lk is None:
                    lk = lqp.tile([128, KVT * CH], BF16, tag="lk", name="lk")
                for m in range(KVT):
                    nc.sync.dma_start(
                        lk[:, m * CH:(m + 1) * CH],
                        kvlat[:, (c * KVT + m) * CH:(c * KVT + m + 1) * CH])
                return lq, lk

            # ---- DMA priority order: interleave the chunk-0 latents with
            # the up-proj weights they are contracted against, so the first
            # matmul starts after ~0.2MB of traffic instead of everything.
            wqn_s = wp.tile([128, QLT * HPC * NOPE], BF16, tag="wqn")
            lq0 = lqp.tile([128, QLT * CH], BF16, tag="lq", name="lq")
            for m in range(QLT):
                nc.sync.dma_start(
                    wqn_s[:, m * HPC * NOPE:(m + 1) * HPC * NOPE],
                    wqn[:, m * HPC * NOPE:(m + 1) * HPC * NOPE])
                nc.sync.dma_start(lq0[:, m * CH:(m + 1) * CH],
                                  qlat[:, m * CH:(m + 1) * CH])
            wqr_s = wp.tile([128, QLT * HPC * 64], BF16, tag="wqr")
            nc.sync.dma_start(wqr_s[:], wqr[:, :])
            wkn_s = wp.tile([128, KVT * HPC * NOPE], BF16, tag="wkn")
            nc.sync.dma_start(wkn_s[:], wkn[:, :])
            lk0 = lqp.tile([128, KVT * CH], BF16, tag="lk", name="lk")
            for m in range(KVT):
                nc.sync.dma_start(lk0[:, m * CH:(m + 1) * CH],
                                  kvlat[:, m * CH:(m + 1) * CH])
            wkv_s = wp.tile([128, KVT * HPC * VH], BF16, tag="wkv")
            nc.sync.dma_start(wkv_s[:], wkv[:, :])
            pend = (lq0, lk0)
            # ---- rope keys / mask band / o-proj weights ----
            nc.sync.dma_start(krs[0][:], kr0[:, :])
            nc.sync.dma_start(krs[1][:], kr1[:, :])
            mt = wp.tile([128, max(n_mask, 1) * QC], BF16, tag="mask")
            nc.sync.dma_start(mt[:], mask[:, :])
            wo_s = wp.tile([128, HPC * HID], BF16, tag="wo")
            nc.sync.dma_start(wo_s[:], wo[:, :])

            # ---- phase 1: up-projections, chunked over tokens ----
            for c in range(NCH):
                csl = slice(c * CH, (c + 1) * CH)
                lq, lk = pend
                if c + 1 < NCH:
                    pend = load_chunk(c + 1)

                for h in range(HPC):
                    ps = pp.tile([128, CH], F32, tag="ups")
                    for m in range(QLT):
                        nc.tensor.matmul(
                            ps[:],
                            wqn_s[:, m * HPC * NOPE + h * NOPE:
                                  m * HPC * NOPE + (h + 1) * NOPE],
                            lq[:, m * CH:(m + 1) * CH],
                            start=(m == 0), stop=(m == QLT - 1))
                    nc.vector.tensor_copy(qn_T[h][:, csl], ps[:])
                ps = pp.tile([128, CH], F32, tag="ups")
                for m in range(QLT):
                    nc.tensor.matmul(ps[:],
                                     wqr_s[:, m * HPC * 64:(m + 1) * HPC * 64],
                                     lq[:, m * CH:(m + 1) * CH],
                                     start=(m == 0), stop=(m == QLT - 1))
                nc.vector.tensor_copy(qr2_T[:, csl], ps[:])
                for h in range(HPC):
                    ps = pp.tile([128, CH], F32, tag="ups")
                    for m in range(KVT):
                        nc.tensor.matmul(
                            ps[:],
                            wkn_s[:, m * HPC * NOPE + h * NOPE:
                                  m * HPC * NOPE + (h + 1) * NOPE],
                            lk[:, m * CH:(m + 1) * CH],
                            start=(m == 0), stop=(m == KVT - 1))
                    nc.vector.tensor_copy(kn_T[h][:, csl], ps[:])
                for st in range(CH // 128):
                    ps = pp.tile([128, HPC * VH], F32, tag="ups")
                    for m in range(KVT):
                        nc.tensor.matmul(
                            ps[:],
                            lk[:, m * CH + st * 128:m * CH + (st + 1) * 128],
                            wkv_s[:, m * HPC * VH:(m + 1) * HPC * VH],
                            start=(m == 0), stop=(m == KVT - 1))
                    gst = c * (CH // 128) + st
                    nc.vector.tensor_copy(
                        v2[:, gst * HPC * VH:(gst + 1) * HPC * VH], ps[:])

            # ---- phase 2: attention per query chunk, heads interleaved ----
            for qc in range(NQC):
                qsl = slice(qc * QC, (qc + 1) * QC)
                tiles = struct[qc]
                nk = len(tiles)
                ps_o = [pop.tile([128, QC], F32, tag="po", name=f"po{h}")
                        for h in range(HPC)]
                esum = [esp.tile([128, QC], BF16, tag="esum",
                                 name=f"esum{h}") for h in range(HPC)]
                for ki, (kt, midx) in enumerate(tiles):
                    ksl = slice(kt * 128, (kt + 1) * 128)
                    for h in range(HPC):
                        ps_s = pp.tile([128, QC], F32, tag="ups",
                                       name="ps_s")
                        nc.tensor.matmul(ps_s[:], kn_T[h][:, ksl],
                                         qn_T[h][:, qsl],
                                         start=True, stop=False)
                        nc.tensor.matmul(ps_s[:], krs[h][:, ksl],
                                         qr2_T[:, qsl],
                                         start=False, stop=True)
                        if midx >= 0:
                            nc.vector.tensor_add(
                                ps_s[:], ps_s[:],
                                mt[:, midx * QC:(midx + 1) * QC])
                        et = ep.tile([128, QC], BF16, tag="et", name="et")
                        nc.scalar.activation(
                            et[:], ps_s[:],
                            mybir.ActivationFunctionType.Exp,
                            bias=zb[:], scale=1.0)
                        if ki == 0:
                            nc.vector.tensor_copy(esum[h][:], et[:])
                        else:
                            nc.vector.tensor_add(esum[h][:], esum[h][:],
                                                 et[:])
                        nc.tensor.matmul(
                            ps_o[h][:],
                            v2[:, kt * HPC * VH + h * VH:
                               kt * HPC * VH + (h + 1) * VH],
                            et[:], start=(ki == 0), stop=(ki == nk - 1))
                ps_den = []
                for h in range(HPC):
                    pd = pdp.tile([1, QC], F32, tag="den", name=f"den{h}")
                    nc.tensor.matmul(pd[:], ones_b[:], esum[h][:],
                                     start=True, stop=True)
                    ps_den.append(pd)
                for h in range(HPC):
                    dencp = tp.tile([1, QC], F32, tag="dencp")
                    nc.scalar.copy(dencp[:], ps_den[h][:])
                    rd = tp.tile([1, QC], F32, tag="rd")
                    nc.vector.reciprocal_approx_fast(rd[:], dencp[:])
                    rdb = tp.tile([128, QC], F32, tag="rdb")
                    nc.gpsimd.partition_broadcast(rdb[:], rd[:1])
                    nc.vector.tensor_mul(ot[qc * HPC + h][:], ps_o[h][:],
                                         rdb[:])

            # ---- phase 3: o-proj from all stored attention outputs ----
            # evacuation copies alternate scalar/vector so neither engine
            # paces the PE; 4 psum tiles stage into one wide row for a
            # single 4KB-per-line DMA.
            k = 0
            for qc in range(NQC):
                for st in range(QC // 128):
                    fo = op.tile([128, HID], BF16, tag="fo")
                    for nn in range(HID // 512):
                        ps_f = pop.tile([128, 512], F32, tag="po",
                                        name="ps_f")
                        for h in range(HPC):
                            nc.tensor.matmul(
                                ps_f[:],
                                ot[qc * HPC + h][:, st * 128:(st + 1) * 128],
                                wo_s[:, h * HID + nn * 512:
                                     h * HID + (nn + 1) * 512],
                                start=(h == 0), stop=(h == HPC - 1))
                        dst = fo[:, nn * 512:(nn + 1) * 512]
                        if k % 2 == 0:
                            nc.scalar.copy(dst, ps_f[:])
                        else:
                            nc.vector.tensor_copy(dst, ps_f[:])
                        k += 1
                    nc.sync.dma_start(
                        part[qc * QC + st * 128:qc * QC + (st + 1) * 128, :],
                        fo[:])
    nc.compile()
    return nc


def _get_a():
    if "a" not in _CACHE:
        _CACHE["a"] = _build_a()
    return _CACHE["a"]


def _get_b(struct, n_mask):
    key = ("b", struct, n_mask)
    if key not in _CACHE:
        _CACHE[key] = _build_b(struct, n_mask)
    return _CACHE[key]


def _analyze_mask(mask_qk):
    """Classify each (qc, kt) block of mask[q, k]; dedupe mixed tiles.

    Returns (struct, mask_tiles [128, n*QC] bf16-able f32 array).
    struct[qc] = tuple of (kt, mask_idx) for k-tiles to visit;
    mask_idx -1 means no mask add needed; fully-masked tiles dropped.
    """
    import ml_dtypes
    uniq = {}
    tiles = []
    struct = []
    for qc in range(NQC):
        row = []
        for kt in range(ST):
            blk = mask_qk[qc * QC:(qc + 1) * QC, kt * 128:(kt + 1) * 128]
            if np.all(blk <= -1e8):
                continue
            if np.all(blk == 0.0):
                row.append((kt, -1))
                continue
            t = np.ascontiguousarray(blk.T).astype(ml_dtypes.bfloat16)
            key = t.tobytes()
            if key not in uniq:
                uniq[key] = len(tiles)
                tiles.append(t)
            row.append((kt, uniq[key]))
        struct.append(tuple(row))
    if tiles:
        mask_tiles = np.concatenate(tiles, axis=1)
    else:
        mask_tiles = np.zeros((128, QC), dtype=ml_dtypes.bfloat16)
    return tuple(struct), mask_tiles


def _prep(hidden_states, attention_mask, Wqa, gqa, Wqb, Wkva, gkva, Wkvb, Wo):
    import ml_dtypes
    bf = ml_dtypes.bfloat16
    f = np.float32
    hid_T = np.ascontiguousarray(hidden_states[0].T, dtype=f)  # [HID, S]
    struct, mask_tiles = _analyze_mask(np.asarray(attention_mask[0, 0], f))
    Wqb_g = (np.asarray(gqa, f)[:, None] * np.asarray(Wqb, f)).astype(f)
    Wkvb_g = (np.asarray(gkva, f)[:, None] * np.asarray(Wkvb, f)).astype(f)

    Wqa_f = np.asarray(Wqa, f)
    wqa_t = Wqa_f.reshape(HT, 128, QLT, 128).transpose(1, 2, 0, 3) \
        .reshape(128, QLT * HT * 128).astype(bf)
    Wkva_f = np.asarray(Wkva, f)
    wkva_t = Wkva_f[:, :KVL].reshape(HT, 128, KVT, 128) \
        .transpose(1, 2, 0, 3).reshape(128, KVT * HT * 128).astype(bf)
    wkvr_t = Wkva_f[:, KVL:].reshape(HT, 128, ROPE).transpose(1, 0, 2) \
        .reshape(128, HT * ROPE).astype(bf)

    ins_a, ins_b = [], []
    for c in range(NCORES):
        hsl = hid_T[:, c * SL:(c + 1) * SL]  # [HID, SL]
        hsl_t = hsl.reshape(HT, 128, SL).transpose(1, 0, 2) \
            .reshape(128, HT * SL).astype(bf)
        ins_a.append({
            "hsl": np.ascontiguousarray(hsl_t),
            "wqa": wqa_t, "wkva": wkva_t, "wkvr": wkvr_t,
        })
        heads = [HPC * c + h for h in range(HPC)]
        wqn = np.concatenate([Wqb_g[:, h * 192:h * 192 + NOPE]
                              for h in heads], axis=1)      # [QL, 256]
        wqr = np.concatenate([Wqb_g[:, h * 192 + NOPE:(h + 1) * 192]
                              for h in heads], axis=1)      # [QL, 128]
        wkn = np.concatenate([Wkvb_g[:, h * 256:h * 256 + NOPE]
                              for h in heads], axis=1)      # [KVL, 256]
        wkv = np.concatenate([Wkvb_g[:, h * 256 + NOPE:(h + 1) * 256]
                              for h in heads], axis=1)      # [KVL, 256]
        wo = np.concatenate([np.asarray(Wo, f)[h * VH:(h + 1) * VH, :]
                             for h in heads], axis=0)       # [256, HID]
        ins_b.append({
            "mask": mask_tiles,
            "wqn": wqn.reshape(QLT, 128, HPC * NOPE).transpose(1, 0, 2)
                      .reshape(128, QLT * HPC * NOPE).astype(bf),
            "wqr": wqr.reshape(QLT, 128, HPC * 64).transpose(1, 0, 2)
                      .reshape(128, QLT * HPC * 64).astype(bf),
            "wkn": wkn.reshape(KVT, 128, HPC * NOPE).transpose(1, 0, 2)
                      .reshape(128, KVT * HPC * NOPE).astype(bf),
            "wkv": wkv.reshape(KVT, 128, HPC * VH).transpose(1, 0, 2)
                      .reshape(128, KVT * HPC * VH).astype(bf),
            "wo": wo.reshape(HPC, 128, HID).transpose(1, 0, 2)
                    .reshape(128, HPC * HID).astype(bf),
        })
    n_mask = mask_tiles.shape[1] // QC
    return ins_a, ins_b, struct, n_mask


def _run(ins_a, ins_b, struct, n_mask, trace=False):
    import ml_dtypes
    bf = ml_dtypes.bfloat16
    core_ids = list(range(NCORES))
    res_a = run_bass_kernel_spmd(_get_a(), ins_a, core_ids, trace=trace)
    qlat = np.concatenate(
        [res_a.results[c]["q_lat"].reshape(128, QLT, SL).transpose(1, 0, 2)
         .reshape(QL, SL) for c in range(NCORES)], axis=1)   # [QL, S] bf16
    kvlat = np.concatenate(
        [res_a.results[c]["kv_lat"].reshape(128, KVT, SL).transpose(1, 0, 2)
         .reshape(KVL, SL) for c in range(NCORES)], axis=1)  # [KVL, S]
    rplat = np.concatenate([res_a.results[c]["rp_lat"]
                            for c in range(NCORES)], axis=1)  # [ROPE, S]
    qlat_t = np.ascontiguousarray(
        qlat.reshape(QLT, 128, NCH, CH).transpose(1, 2, 0, 3)
        .reshape(128, NCH * QLT * CH))
    kvlat_t = np.ascontiguousarray(
        kvlat.reshape(KVT, 128, NCH, CH).transpose(1, 2, 0, 3)
        .reshape(128, NCH * KVT * CH))
    kr0 = np.zeros((128, S), dtype=bf)
    kr0[:ROPE] = rplat
    kr1 = np.zeros((128, S), dtype=bf)
    kr1[ROPE:] = rplat
    for m in ins_b:
        m["qlat"] = qlat_t
        m["kvlat"] = kvlat_t
        m["kr0"] = kr0
        m["kr1"] = kr1
    res_b = run_bass_kernel_spmd(_get_b(struct, n_mask), ins_b, core_ids,
                                 trace=trace)
    out = res_b.results[0]["part"].astype(np.float32)
    for c in range(1, NCORES):
        out = out + res_b.results[c]["part"].astype(np.float32)
    return out[None], res_a, res_b


def kernel(hidden_states, attention_mask, Wqa, gqa, Wqb, Wkva, gkva, Wkvb, Wo):
    ins_a, ins_b, struct, n_mask = _prep(hidden_states, attention_mask, Wqa,
                                         gqa, Wqb, Wkva, gkva, Wkvb, Wo)
    out, _, _ = _run(ins_a, ins_b, struct, n_mask)
    return out
